# revision 26
# baseline (speedup 1.0000x reference)
"""Fused LayerNorm + causal multi-head attention + output projection for
Trainium2, distributed over 8 NeuronCores.

Problem (full shapes): x [4, 2048, 1024], g_ln [1024], Wq [1024, 1024],
Wkv [1024, 2048], Wo [1024, 1024]; B=4, N=2048, D=1024, H=16, DH=64.

Sharding: DP(batch)=4 x TP(heads)=2. Core c handles batch b=c//2 and head
group g=c%2 (heads [g*8, g*8+8)). Each core computes LN(x_b), projects
q/k/v for its 8 heads (g_ln and the 1/sqrt(DH) scale are folded into the
weights host-side), runs causal attention, and multiplies by its slice of
Wo rows, producing a partial [2048, 1024] output. The host sums the two
partials per batch (row-parallel Wo reduce done on host).

On-chip layout notes:
 - All matmuls run as float32r (full-rate fp32 path on the PE).
 - Scores are computed transposed (S^T[j, i]) so softmax denominators come
   from a ones-column appended to V, and no P transposes are needed.
 - Causal masking multiplies the post-exp diagonal [128,128] block by a
   binary lower-triangle; fully-masked columns left of the diagonal are
   skipped entirely (trimmed QK/exp/PV ranges).
"""

import sys

for _p in ("/opt/trn_rl_repo",):
    if _p not in sys.path:
        sys.path.insert(0, _p)

import numpy as np

import concourse.bacc as bacc
import concourse.mybir as mybir
import concourse.tile as tile
from concourse.bass_utils import run_bass_kernel_spmd

N_CORES = 8
B, N, D, H = 4, 2048, 1024, 16
DH = D // H
HL = 8  # heads per core
EPS = 1e-5
F32 = mybir.dt.float32
F32R = mybir.dt.float32r


def build_module(repeat: int = 1):
    nc = bacc.Bacc("TRN2", target_bir_lowering=False)

    x_h = nc.dram_tensor("x", [N, D], F32, kind="ExternalInput")
    wq_h = nc.dram_tensor("wq", [D, 512], F32R, kind="ExternalInput")
    wk_h = nc.dram_tensor("wk", [D, 512], F32R, kind="ExternalInput")
    wv_h = nc.dram_tensor("wv", [D, 512], F32R, kind="ExternalInput")
    wo_h = nc.dram_tensor("wo", [512, D], F32R, kind="ExternalInput")
    tri_h = nc.dram_tensor("tri", [128, 128], F32, kind="ExternalInput")
    ident_h = nc.dram_tensor("ident", [128, 128], F32, kind="ExternalInput")
    out_h = nc.dram_tensor("out", [N, D], F32, kind="ExternalOutput")

    with tile.TileContext(nc) as tc:

        def body(_iv=None):
            _body(nc, tc, x_h, wq_h, wk_h, wv_h, wo_h, tri_h, ident_h, out_h)

        if repeat == 1:
            body()
        else:
            with tc.For_i(0, repeat, 1):
                body()

    nc.compile()
    return nc


def _body(nc, tc, x_h, wq_h, wk_h, wv_h, wo_h, tri_h, ident_h, out_h):
    from contextlib import ExitStack

    with ExitStack() as ctx:
        persist = ctx.enter_context(tc.tile_pool(name="persist", bufs=1))

        identsb = persist.tile([128, 128], F32)
        nc.sync.dma_start(out=identsb, in_=ident_h[:, :])
        trisb = persist.tile([128, 128], F32)
        nc.sync.dma_start(out=trisb, in_=tri_h[:, :])

        qT = persist.tile([128, 4, N], F32R)
        kT = persist.tile([128, 4, N], F32R)
        vsc = persist.tile([128, 16, HL, 65], F32R)
        OTsb = persist.tile([128, 4, N], F32R)

        # ---------------- Phase A+B: LayerNorm -> xn^T, projections -------
        with ExitStack() as ab:
            abp = ab.enter_context(tc.tile_pool(name="abp", bufs=1))
            lnp = ab.enter_context(tc.tile_pool(name="lnp", bufs=2))
            wsp = ab.enter_context(tc.tile_pool(name="wsp", bufs=2))
            psA = ab.enter_context(tc.tile_pool(name="psA", bufs=3, space="PSUM"))
            psB = ab.enter_context(tc.tile_pool(name="psB", bufs=2, space="PSUM"))

            eps_t = abp.tile([128, 1], F32)
            nc.vector.memset(eps_t, EPS)
            ones8 = abp.tile([128, 8], F32)
            nc.vector.memset(ones8, 1.0)

            wv_sb = abp.tile([128, 8, 512], F32R)

            xnT_q = [None] * 4
            for th in range(4):  # token quarters, double-buffered xnT
                t0 = th * 512
                xnT = abp.tile([128, 8, 512], F32R, tag="xnT", bufs=2)
                xnT_q[th] = xnT

                for tt in range(4):
                    xt = lnp.tile([128, D], F32, tag="xt", bufs=3)
                    nc.sync.dma_start(
                        out=xt, in_=x_h[t0 + tt * 128 : t0 + (tt + 1) * 128, :]
                    )
                    st = lnp.tile([128, 2, 6], F32, tag="st")
                    for sg in range(2):
                        nc.vector.bn_stats(
                            out=st[:, sg, :], in_=xt[:, sg * 512 : (sg + 1) * 512]
                        )
                    mv = lnp.tile([128, 2], F32, tag="mv")
                    nc.vector.bn_aggr(out=mv, in_=st)
                    rs = lnp.tile([128, 1], F32, tag="rs")
                    nc.scalar.activation(
                        out=rs, in_=mv[:, 1:2],
                        func=mybir.ActivationFunctionType.Sqrt,
                        bias=eps_t, scale=1.0,
                    )
                    nc.vector.reciprocal(out=rs, in_=rs)
                    nc.vector.tensor_scalar(
                        out=xt, in0=xt, scalar1=mv[:, 0:1], scalar2=rs,
                        op0=mybir.AluOpType.subtract, op1=mybir.AluOpType.mult,
                    )
                    for grp in range(2):
                        trp = psA.tile([128, 4, 128], F32, tag="trp")
                        for j in range(4):
                            dk = grp * 4 + j
                            nc.tensor.matmul(
                                trp[:, j, :],
                                lhsT=xt[:, dk * 128 : (dk + 1) * 128],
                                rhs=identsb, is_transpose=True,
                                start=True, stop=True,
                            )
                        nc.scalar.copy(
                            out=xnT[:, grp * 4 : grp * 4 + 4, tt * 128 : (tt + 1) * 128],
                            in_=trp,
                        )

                if th == 0:
                    nc.sync.dma_start(
                        out=wv_sb,
                        in_=wv_h[:, :].rearrange("(dk r) m -> r dk m", r=128),
                    )

                # v projection for this quarter
                for tt in range(4):
                    psv = psB.tile([128, 512], F32, tag="psv")
                    for dk in range(8):
                        nc.tensor.matmul(
                            psv, lhsT=xnT[:, dk, tt * 128 : (tt + 1) * 128],
                            rhs=wv_sb[:, dk, :],
                            start=(dk == 0), stop=(dk == 7),
                        )
                    jt = th * 4 + tt
                    nc.vector.tensor_copy(
                        out=vsc[:, jt, :, 0:64],
                        in_=psv.rearrange("r (h d) -> r h d", h=HL),
                    )
                    nc.vector.tensor_copy(
                        out=vsc[:, jt, :, 64:65].rearrange("p h o -> p (h o)"),
                        in_=ones8,
                    )

                # q/k projections once per half, reading both live quarters
                if th % 2 == 1:
                    h0 = (th - 1) * 512
                    for p in range(4):
                        wqs = wsp.tile([128, 8, 128], F32R, tag="wqs")
                        nc.sync.dma_start(
                            out=wqs,
                            in_=wq_h[:, p * 128 : (p + 1) * 128].rearrange(
                                "(dk r) m -> r dk m", r=128
                            ),
                        )
                        wks = wsp.tile([128, 8, 128], F32R, tag="wks")
                        nc.sync.dma_start(
                            out=wks,
                            in_=wk_h[:, p * 128 : (p + 1) * 128].rearrange(
                                "(dk r) m -> r dk m", r=128
                            ),
                        )
                        for t4 in range(2):
                            xq = xnT_q[th - 1 + t4]
                            psq = psB.tile([128, 512], F32, tag="pqk")
                            for dk in range(8):
                                nc.tensor.matmul(
                                    psq, lhsT=wqs[:, dk, :],
                                    rhs=xq[:, dk, :],
                                    start=(dk == 0), stop=(dk == 7),
                                )
                            nc.scalar.copy(
                                out=qT[:, p, h0 + t4 * 512 : h0 + (t4 + 1) * 512],
                                in_=psq,
                            )
                            psk = psB.tile([128, 512], F32, tag="pqk")
                            for dk in range(8):
                                nc.tensor.matmul(
                                    psk, lhsT=wks[:, dk, :],
                                    rhs=xq[:, dk, :],
                                    start=(dk == 0), stop=(dk == 7),
                                )
                            nc.scalar.copy(
                                out=kT[:, p, h0 + t4 * 512 : h0 + (t4 + 1) * 512],
                                in_=psk,
                            )

        # ---- Phase C/D interleaved: attention per i-half, then that
        # half's output projection (psS 4 + psO 2 + psD 2 = 8 banks) -------
        dsp = ctx.enter_context(tc.tile_pool(name="dsp", bufs=1))
        outp = ctx.enter_context(tc.tile_pool(name="outp", bufs=4))
        psD = ctx.enter_context(tc.tile_pool(name="psD", bufs=2, space="PSUM"))
        wo_sb = dsp.tile([128, 4, D], F32R)
        nc.sync.dma_start(
            out=wo_sb, in_=wo_h[:, :].rearrange("(ck r) e -> r ck e", r=128)
        )

        def wo_half(ihalf, pool):
            for tt in range(ihalf * 8, ihalf * 8 + 8):
                for e2 in range(2):
                    pso = pool.tile([128, 512], F32, tag="pso")
                    for ck in range(4):
                        nc.tensor.matmul(
                            pso, lhsT=OTsb[:, ck, tt * 128 : (tt + 1) * 128],
                            rhs=wo_sb[:, ck, e2 * 512 : (e2 + 1) * 512],
                            start=(ck == 0), stop=(ck == 3),
                        )
                    osb = outp.tile([128, 512], F32, tag="osb")
                    nc.vector.tensor_copy(out=osb, in_=pso)
                    nc.sync.dma_start(
                        out=out_h[tt * 128 : (tt + 1) * 128,
                                  e2 * 512 : (e2 + 1) * 512],
                        in_=osb,
                    )

        with ExitStack() as cs:
            expp = cs.enter_context(tc.tile_pool(name="expp", bufs=3))
            denp = cs.enter_context(tc.tile_pool(name="denp", bufs=3))
            drp = cs.enter_context(tc.tile_pool(name="drp", bufs=2, space="DRAM"))
            psS = cs.enter_context(tc.tile_pool(name="psS", bufs=2, space="PSUM"))
            psO = cs.enter_context(tc.tile_pool(name="psO", bufs=1, space="PSUM"))

            for ihalf in range(2):
                half0 = ihalf * 1024
                for p in range(4):
                    for hh in range(2):
                        h = p * 2 + hh
                        row0 = hh * 64
                        OTp = psO.tile([128, 1024], F32, tag="OTp")
                        for ji in range(8 if ihalf == 0 else 16):
                            dt_i = (ji * 128) // 512 * 512
                            i_lo = max(half0, dt_i)
                            W = half0 + 1024 - i_lo
                            nblk = W // 512
                            d = 0
                            has_mask = dt_i >= half0
                            if has_mask:
                                d = ji * 128 - dt_i
                            Sp = psS.tile([128, 1024], F32, tag="Sp")
                            for s5 in range(nblk):
                                lo = d if s5 == 0 else 0
                                nc.tensor.matmul(
                                    Sp[:, s5 * 512 + lo : (s5 + 1) * 512],
                                    lhsT=kT[row0 : row0 + 64, p,
                                            ji * 128 : (ji + 1) * 128],
                                    rhs=qT[row0 : row0 + 64, p,
                                           i_lo + s5 * 512 + lo : i_lo + (s5 + 1) * 512],
                                    start=True, stop=True,
                                )
                            expS = expp.tile([128, 1024], F32R, tag="expS")
                            nc.scalar.activation(
                                out=expS[:, d:W], in_=Sp[:, d:W],
                                func=mybir.ActivationFunctionType.Exp,
                            )
                            if has_mask:
                                nc.vector.tensor_tensor(
                                    out=expS[:, d : d + 128],
                                    in0=expS[:, d : d + 128], in1=trisb,
                                    op=mybir.AluOpType.mult,
                                )
                            for s5 in range(nblk):
                                blk_i = i_lo + s5 * 512
                                off = blk_i - half0
                                lo = d if s5 == 0 else 0
                                nc.tensor.matmul(
                                    OTp[0:65, off + lo : off + 512],
                                    lhsT=vsc[:, ji, h, :],
                                    rhs=expS[:, s5 * 512 + lo : (s5 + 1) * 512],
                                    start=(ji == 0), stop=(ji == blk_i // 128 + 3),
                                )
                        # copy out early (releases the OT psum slot), then
                        # normalize rows 0..63 by 1/rowsum (row 64).
                        cp = denp.tile([128, 1024], F32, tag="cp")
                        nc.vector.tensor_copy(out=cp[0:65, :], in_=OTp[0:65, :])
                        nc.vector.reciprocal(out=cp[64:65, :], in_=cp[64:65, :])
                        dscr = drp.tile([1, 1024], F32, tag="dscr")
                        nc.sync.dma_start(out=dscr, in_=cp[64:65, :])
                        bc = denp.tile([128, 1024], F32, tag="bc")
                        nc.sync.dma_start(
                            out=bc[0:64, :], in_=dscr.broadcast_to((64, 1024))
                        )
                        nc.vector.tensor_tensor(
                            out=cp[0:64, :], in0=cp[0:64, :], in1=bc[0:64, :],
                            op=mybir.AluOpType.mult,
                        )
                        nc.sync.dma_start(
                            out=OTsb[row0 : row0 + 64, p, half0 : half0 + 1024],
                            in_=cp[0:64, :].bitcast(F32R),
                        )

                if ihalf == 0:
                    wo_half(0, psD)

        # attention pools closed: 6 banks free for a deep final Wo pipeline
        psD2 = ctx.enter_context(tc.tile_pool(name="psD2", bufs=6, space="PSUM"))
        wo_half(1, psD2)


_CACHE = {}


def _get_module(repeat: int = 1):
    if repeat not in _CACHE:
        _CACHE[repeat] = build_module(repeat)
    return _CACHE[repeat]


def _make_tri():
    r = np.arange(128)[:, None]
    c = np.arange(128)[None, :]
    return (c >= r).astype(np.float32)  # 1 = attend (j <= i), 0 = masked


def _prep_in_maps(x, g_ln, Wq, Wkv, Wo):
    x = np.asarray(x, dtype=np.float32)
    g_ln = np.asarray(g_ln, dtype=np.float32)
    Wq = np.asarray(Wq, dtype=np.float32)
    Wkv = np.asarray(Wkv, dtype=np.float32)
    Wo = np.asarray(Wo, dtype=np.float32)

    scale = np.float32(DH ** -0.5)
    wq_full = (g_ln[:, None] * Wq * scale).astype(np.float32)
    wk_full = (g_ln[:, None] * Wkv[:, :D]).astype(np.float32)
    wv_full = (g_ln[:, None] * Wkv[:, D:]).astype(np.float32)

    tri = _make_tri()
    ident = np.eye(128, dtype=np.float32)

    in_maps = []
    for c in range(N_CORES):
        b, g = c // 2, c % 2
        sl = slice(g * 512, (g + 1) * 512)
        in_maps.append(
            {
                "x": np.ascontiguousarray(x[b]),
                "wq": np.ascontiguousarray(wq_full[:, sl]),
                "wk": np.ascontiguousarray(wk_full[:, sl]),
                "wv": np.ascontiguousarray(wv_full[:, sl]),
                "wo": np.ascontiguousarray(Wo[sl, :]),
                "tri": tri,
                "ident": ident,
            }
        )
    return in_maps


def kernel(x, g_ln, Wq, Wkv, Wo):
    nc = _get_module(repeat=1)
    in_maps = _prep_in_maps(x, g_ln, Wq, Wkv, Wo)
    res = run_bass_kernel_spmd(nc, in_maps, list(range(N_CORES)))
    out = np.empty((B, N, D), dtype=np.float32)
    for b in range(B):
        out[b] = res.results[2 * b]["out"] + res.results[2 * b + 1]["out"]
    return out
